# Initial kernel scaffold
#
"""Trainium2 Bass kernel for nn_CapsuleLayer_9852654977072.

The reference module collapses mathematically: the routing loop's coupling
logits `b` stay zero (faithfully-reproduced bug in the original torch code),
so routing coefficients are a fixed spatial map r(h,w) = 1/(8*cnt(h,w)) where
cnt is the 5x5 box-count inside the image. The whole module is therefore:

    p = conv2d(u as [N,64,H,W], Wd as [128,64,5,5], pad=2) * s(h,w)
    v = squash_z1(p)   # groups of 16 channels
    out[n,t1,z1,h,w] = v

Device strategy (8 cores, SPMD): shard (batch n in 0..3) x (row-half in 0..1).
Each core computes all 128 output channels for 64 rows of one image.
Conv = 13 PSUM-accumulated matmuls per 4-row block (taps packed in pairs on
the 128-partition contraction axis; inputs shipped with a +1-row shifted copy
XA and a +1-col shifted copy XC so each fp32r matmul covers 2 taps at full
PE rate). Squash via square (ACT) -> block-diag matmul (n2) -> factor on
8 partitions -> expand matmul -> multiply.
"""

import numpy as np

T0, Z0, T1, Z1, KK, PAD = 4, 16, 8, 16, 5, 2
N, H, W_SP = 4, 128, 128
CIN, COUT = T0 * Z0, T1 * Z1  # 64, 128
N_CORES = 8
ROWS = 64          # output rows per core
XROWS = 68         # input rows incl. halo
XCOLS = 132        # 128 + 2*PAD
N_MM = 13
BLK = 4            # output rows per block
N_BLKS = ROWS // BLK

# (source, row_off, col_off) per matmul j; weights built to match in _weight_tiles
_MM_SLICES = (
    [('XA', dy + 2, dx + 2) for dy in (-2, 0) for dx in (-2, -1, 0, 1, 2)]
    + [('XC', 2, 0), ('XC', 2, 2), ('XC', 2, 4)]
)

_CACHE = {}


def _weight_tiles(W):
    Wd = W.transpose(1, 0, 2, 3, 4).reshape(COUT, CIN, KK, KK)
    wl = np.zeros((128, N_MM, 128), np.float32)  # [k, j, m]
    j = 0
    for dy in (-2, 0):
        for dx in (-2, -1, 0, 1, 2):
            wl[0:64, j, :] = Wd[:, :, dy + 2, dx + 2].T
            wl[64:128, j, :] = Wd[:, :, dy + 3, dx + 2].T
            j += 1
    for dx0 in (-2, 0):
        wl[0:64, j, :] = Wd[:, :, 4, dx0 + 2].T
        wl[64:128, j, :] = Wd[:, :, 4, dx0 + 3].T
        j += 1
    wl[0:64, j, :] = Wd[:, :, 4, 4].T  # single tap (2,2); half1 zero
    return wl


def _inputs_core(x, half):
    """x: [64, H, W] one image channel-major. Returns XA, XC [128, 68, 132]."""
    base = half * 64 - 2
    XA = np.zeros((128, XROWS, XCOLS), np.float32)
    XC = np.zeros((128, XROWS, XCOLS), np.float32)

    def fill(dst, roff, c0, c1):
        lo, hi = max(0, -(base + roff)), min(XROWS, H - base - roff)
        dst[:, lo:hi, c0:c1] = x[:, base + roff + lo:base + roff + hi, :]

    fill(XA[0:64], 0, 2, 130)
    fill(XA[64:128], 1, 2, 130)
    fill(XC[0:64], 2, 2, 130)
    fill(XC[64:128], 2, 1, 129)
    return XA, XC


def _s_map():
    idx = np.arange(H)
    cnt = (np.minimum(idx + 2, H - 1) - np.maximum(idx - 2, 0) + 1).astype(np.float32)
    return 1.0 / (8.0 * cnt[:, None] * cnt[None, :])  # [H, W]


def _block_diag():
    bd = np.zeros((128, 8), np.float32)
    bd[np.arange(128), np.arange(128) // 16] = 1.0
    return bd


def build_nc():
    import concourse.bass as bass
    import concourse.mybir as mybir
    import concourse.tile as tile

    f32 = mybir.dt.float32
    f32r = mybir.dt.float32r
    AF = mybir.ActivationFunctionType

    nc = bass.Bass(target_bir_lowering=False)
    xa_d = nc.dram_tensor("xa", [128, XROWS * XCOLS], f32, kind="ExternalInput")
    xc_d = nc.dram_tensor("xc", [128, XROWS * XCOLS], f32, kind="ExternalInput")
    wl_d = nc.dram_tensor("wl", [128, N_MM * 128], f32, kind="ExternalInput")
    bd_d = nc.dram_tensor("bd", [128, 8], f32, kind="ExternalInput")
    ex_d = nc.dram_tensor("ex", [8, 128], f32, kind="ExternalInput")
    ss_d = nc.dram_tensor("ss", [1, ROWS * 128], f32, kind="ExternalInput")
    out_d = nc.dram_tensor("out", [128, ROWS * 128], f32, kind="ExternalOutput")

    with tile.TileContext(nc) as tc:
        with (
            tc.tile_pool(name="consts", bufs=1) as consts,
            tc.tile_pool(name="work", bufs=3) as work,
            tc.tile_pool(name="small", bufs=4) as small,
            tc.tile_pool(name="pp", bufs=2, space="PSUM") as pp,
            tc.tile_pool(name="pf", bufs=2, space="PSUM") as pf,
            tc.tile_pool(name="py", bufs=2, space="PSUM") as py,
        ):
            xa = consts.tile([128, XROWS, XCOLS], f32)
            nc.sync.dma_start(
                out=xa, in_=xa_d.ap().rearrange("p (r c) -> p r c", c=XCOLS))
            xc = consts.tile([128, XROWS, XCOLS], f32)
            nc.sync.dma_start(
                out=xc, in_=xc_d.ap().rearrange("p (r c) -> p r c", c=XCOLS))
            wl = consts.tile([128, N_MM, 128], f32)
            nc.sync.dma_start(
                out=wl, in_=wl_d.ap().rearrange("p (j m) -> p j m", m=128))
            bd = consts.tile([128, 8], f32)
            nc.sync.dma_start(out=bd, in_=bd_d.ap())
            ex = consts.tile([8, 128], f32)
            nc.sync.dma_start(out=ex, in_=ex_d.ap())
            # per-pixel scale broadcast to all 128 partitions
            s_sb = consts.tile([128, ROWS, 128], f32)
            ss_ap = ss_d.ap()
            ss_b = bass.AP(
                tensor=ss_ap.tensor, offset=ss_ap.offset,
                ap=[[0, 128], [128, ROWS], [1, 128]])
            nc.sync.dma_start(out=s_sb, in_=ss_b)

            out_v = out_d.ap().rearrange("p (r c) -> p r c", c=128)

            for blk in range(N_BLKS):
                r0 = blk * BLK
                p_ps = pp.tile([128, BLK, 128], f32)
                for j, (src, roff, coff) in enumerate(_MM_SLICES):
                    xsrc = xa if src == 'XA' else xc
                    rhs = xsrc[:, r0 + roff:r0 + roff + BLK, coff:coff + 128]
                    nc.tensor.matmul(
                        p_ps[:],
                        wl[:, j, :].bitcast(f32r),
                        rhs.bitcast(f32r),
                        start=(j == 0), stop=(j == N_MM - 1),
                    )
                psc = work.tile([128, BLK, 128], f32, tag="psc")
                nc.vector.tensor_mul(psc[:], p_ps[:], s_sb[:, r0:r0 + BLK, :])
                psq = work.tile([128, BLK, 128], f32, tag="psq")
                nc.scalar.activation(psq[:], psc[:], AF.Square)
                y_ps = py.tile([8, BLK, 128], f32)
                nc.tensor.matmul(
                    y_ps[:], bd[:].bitcast(f32r), psq[:].bitcast(f32r),
                    start=True, stop=True)
                a_t = small.tile([8, BLK, 128], f32, tag="a")
                nc.scalar.activation(a_t[:], y_ps[:], AF.Sqrt, bias=1e-9)
                y1_t = small.tile([8, BLK, 128], f32, tag="y1")
                nc.scalar.activation(y1_t[:], y_ps[:], AF.Copy, bias=1.0)
                b_t = small.tile([8, BLK, 128], f32, tag="b")
                nc.vector.tensor_mul(b_t[:], a_t[:], y1_t[:])
                r_t = small.tile([8, BLK, 128], f32, tag="r")
                nc.vector.reciprocal(r_t[:], b_t[:])
                F_t = small.tile([8, BLK, 128], f32, tag="F")
                nc.vector.tensor_mul(F_t[:], y_ps[:], r_t[:])
                fe_ps = pf.tile([128, BLK, 128], f32)
                nc.tensor.matmul(
                    fe_ps[:], ex[:].bitcast(f32r), F_t[:].bitcast(f32r),
                    start=True, stop=True)
                v_t = work.tile([128, BLK, 128], f32, tag="v")
                nc.vector.tensor_mul(v_t[:], psc[:], fe_ps[:])
                nc.sync.dma_start(out=out_v[:, r0:r0 + BLK, :], in_=v_t[:])

    nc.compile()
    return nc


def _prep_in_maps(u, W):
    x = u.reshape(N, CIN, H, W_SP)
    wl = _weight_tiles(W).reshape(128, N_MM * 128)
    bd = _block_diag()
    ex = np.ascontiguousarray(bd.T)
    s = _s_map()
    in_maps = []
    for core in range(N_CORES):
        n, half = core // 2, core % 2
        XA, XC = _inputs_core(x[n], half)
        ss = s[half * 64:(half + 1) * 64, :].reshape(1, ROWS * 128)
        in_maps.append({
            "xa": XA.reshape(128, XROWS * XCOLS),
            "xc": XC.reshape(128, XROWS * XCOLS),
            "wl": wl,
            "bd": bd,
            "ex": ex,
            "ss": np.ascontiguousarray(ss),
        })
    return in_maps


def run(u, W, trace=False):
    """Returns (out [N,T1,Z1,H,W] f32, BassKernelResults)."""
    from concourse.bass_utils import run_bass_kernel_spmd

    if "nc" not in _CACHE:
        _CACHE["nc"] = build_nc()
    nc = _CACHE["nc"]
    in_maps = _prep_in_maps(np.asarray(u, np.float32), np.asarray(W, np.float32))
    res = run_bass_kernel_spmd(nc, in_maps, list(range(N_CORES)), trace=trace)
    out = np.empty((N, T1, Z1, H, W_SP), np.float32)
    for core in range(N_CORES):
        n, half = core // 2, core % 2
        o = res.results[core]["out"].reshape(T1, Z1, ROWS, 128)
        out[n, :, :, half * 64:(half + 1) * 64, :] = o
    return out, res


def kernel(u, W):
    out, _ = run(u, W, trace=False)
    return out


# revision 7
# speedup vs baseline: 2.3449x; 2.3449x over previous
"""Trainium2 Bass kernel for nn_CapsuleLayer_9852654977072.

The reference module collapses mathematically: the routing loop's coupling
logits `b` stay zero (faithfully-reproduced bug in the original torch code),
so routing coefficients are a fixed spatial map r(h,w) = 1/(8*cnt(h,w)) where
cnt is the 5x5 box-count inside the image. The whole module is therefore:

    p = conv2d(u as [N,64,H,W], Wd as [128,64,5,5], pad=2) * s(h,w)
    v = squash_z1(p)   # groups of 16 channels
    out[n,t1,z1,h,w] = v

Device strategy (8 cores, SPMD): shard (batch n in 0..3) x (row-half in 0..1).
Each core computes all 128 output channels for 64 rows of one image.
Conv = 13 PSUM-accumulated matmuls per 4-row block (taps packed in pairs on
the 128-partition contraction axis; inputs shipped with a +1-row shifted copy
XA and a +1-col shifted copy XC so each fp32r matmul covers 2 taps at full
PE rate). Squash via square (ACT) -> block-diag matmul (n2) -> factor on
8 partitions -> expand matmul -> multiply.
"""

import numpy as np

T0, Z0, T1, Z1, KK, PAD = 4, 16, 8, 16, 5, 2
N, H, W_SP = 4, 128, 128
CIN, COUT = T0 * Z0, T1 * Z1  # 64, 128
N_CORES = 8
ROWS = 64          # output rows per core
XROWS = 68         # input rows incl. halo
XCOLS = 132        # 128 + 2*PAD
N_MM = 13
BLK = 4            # output rows per block
N_BLKS = ROWS // BLK

# (source, row_off, col_off) per matmul j; weights built to match in _weight_tiles
_MM_SLICES = (
    [('XA', dy + 2, dx + 2) for dy in (-2, 0) for dx in (-2, -1, 0, 1, 2)]
    + [('XC', 2, 0), ('XC', 2, 2), ('XC', 2, 4)]
)

_CACHE = {}


def _weight_tiles(W):
    Wd = W.transpose(1, 0, 2, 3, 4).reshape(COUT, CIN, KK, KK)
    wl = np.zeros((128, N_MM, 128), np.float32)  # [k, j, m]
    j = 0
    for dy in (-2, 0):
        for dx in (-2, -1, 0, 1, 2):
            wl[0:64, j, :] = Wd[:, :, dy + 2, dx + 2].T
            wl[64:128, j, :] = Wd[:, :, dy + 3, dx + 2].T
            j += 1
    for dx0 in (-2, 0):
        wl[0:64, j, :] = Wd[:, :, 4, dx0 + 2].T
        wl[64:128, j, :] = Wd[:, :, 4, dx0 + 3].T
        j += 1
    wl[0:64, j, :] = Wd[:, :, 4, 4].T  # single tap (2,2); half1 zero
    return wl


def _inputs_core(x, half):
    """x: [64, H, W] one image channel-major. Returns XA, XC [128, 68, 132]."""
    base = half * 64 - 2
    XA = np.zeros((128, XROWS, XCOLS), np.float32)
    XC = np.zeros((128, XROWS, XCOLS), np.float32)

    def fill(dst, roff, c0, c1):
        lo, hi = max(0, -(base + roff)), min(XROWS, H - base - roff)
        dst[:, lo:hi, c0:c1] = x[:, base + roff + lo:base + roff + hi, :]

    fill(XA[0:64], 0, 2, 130)
    fill(XA[64:128], 1, 2, 130)
    fill(XC[0:64], 2, 2, 130)
    fill(XC[64:128], 2, 1, 129)
    return XA, XC


def _s_map():
    idx = np.arange(H)
    cnt = (np.minimum(idx + 2, H - 1) - np.maximum(idx - 2, 0) + 1).astype(np.float32)
    return 1.0 / (8.0 * cnt[:, None] * cnt[None, :])  # [H, W]


def _block_diag():
    bd = np.zeros((128, 8), np.float32)
    bd[np.arange(128), np.arange(128) // 16] = 1.0
    return bd


def build_nc(reps=1):
    import concourse.bass as bass
    import concourse.bacc as bacc
    import concourse.mybir as mybir
    import concourse.tile as tile

    f32 = mybir.dt.float32
    f32r = mybir.dt.float32r
    AF = mybir.ActivationFunctionType

    nc = bacc.Bacc(None, target_bir_lowering=False)
    xa_d = nc.dram_tensor("xa", [128, XROWS * XCOLS], f32r, kind="ExternalInput")
    xc_d = nc.dram_tensor("xc", [128, XROWS * XCOLS], f32r, kind="ExternalInput")
    wl_d = nc.dram_tensor("wl", [128, N_MM * 128], f32r, kind="ExternalInput")
    bd_d = nc.dram_tensor("bd", [128, 8], f32r, kind="ExternalInput")
    ex_d = nc.dram_tensor("ex", [8, 128], f32r, kind="ExternalInput")
    ss_d = nc.dram_tensor("ss", [1, ROWS * 128], f32, kind="ExternalInput")
    out_d = nc.dram_tensor("out", [128, ROWS * 128], f32, kind="ExternalOutput")

    with tile.TileContext(nc) as tc:
        with (
            tc.tile_pool(name="consts", bufs=1) as consts,
            tc.tile_pool(name="work", bufs=3) as work,
            tc.tile_pool(name="small", bufs=4) as small,
            tc.tile_pool(name="pp", bufs=2, space="PSUM") as pp,
            tc.tile_pool(name="pf", bufs=2, space="PSUM") as pf,
            tc.tile_pool(name="py", bufs=2, space="PSUM") as py,
        ):
            xa = consts.tile([128, XROWS, XCOLS], f32r)
            nc.sync.dma_start(
                out=xa, in_=xa_d.ap().rearrange("p (r c) -> p r c", c=XCOLS))
            xc = consts.tile([128, XROWS, XCOLS], f32r)
            nc.sync.dma_start(
                out=xc, in_=xc_d.ap().rearrange("p (r c) -> p r c", c=XCOLS))
            wl = consts.tile([128, N_MM, 128], f32r)
            nc.sync.dma_start(
                out=wl, in_=wl_d.ap().rearrange("p (j m) -> p j m", m=128))
            bd = consts.tile([128, 8], f32r)
            nc.sync.dma_start(out=bd, in_=bd_d.ap())
            ex = consts.tile([8, 128], f32r)
            nc.sync.dma_start(out=ex, in_=ex_d.ap())
            # per-pixel scale broadcast to all 128 partitions
            s_sb = consts.tile([128, ROWS, 128], f32)
            ss_ap = ss_d.ap()
            ss_b = bass.AP(
                tensor=ss_ap.tensor, offset=ss_ap.offset,
                ap=[[0, 128], [128, ROWS], [1, 128]])
            nc.sync.dma_start(out=s_sb, in_=ss_b)
            eps_t = consts.tile([8, 1], f32)
            nc.vector.memset(eps_t[:], 1e-9)

            out_v = out_d.ap().rearrange("p (r c) -> p r c", c=128)

            for blk in range(N_BLKS * reps):
                blk %= N_BLKS
                r0 = blk * BLK
                p_ps = pp.tile([128, BLK, 128], f32)
                for j, (src, roff, coff) in enumerate(_MM_SLICES):
                    xsrc = xa if src == 'XA' else xc
                    rhs = xsrc[:, r0 + roff:r0 + roff + BLK, coff:coff + 128]
                    nc.tensor.matmul(
                        p_ps[:],
                        wl[:, j, :],
                        rhs,
                        start=(j == 0), stop=(j == N_MM - 1),
                    )
                psc = work.tile([128, BLK, 128], f32, tag="psc")
                nc.vector.tensor_mul(psc[:], p_ps[:], s_sb[:, r0:r0 + BLK, :])
                psq = work.tile([128, BLK, 128], f32r, tag="psq")
                nc.scalar.activation(psq[:], psc[:], AF.Square)
                y_ps = py.tile([8, BLK, 128], f32)
                nc.tensor.matmul(
                    y_ps[:], bd[:], psq[:],
                    start=True, stop=True)
                a_t = small.tile([8, BLK, 128], f32, tag="a")
                nc.scalar.activation(a_t[:], y_ps[:], AF.Sqrt, bias=eps_t[:])
                y1_t = small.tile([8, BLK, 128], f32, tag="y1")
                nc.scalar.activation(y1_t[:], y_ps[:], AF.Copy, bias=1.0)
                b_t = small.tile([8, BLK, 128], f32, tag="b")
                nc.vector.tensor_mul(b_t[:], a_t[:], y1_t[:])
                r_t = small.tile([8, BLK, 128], f32, tag="r")
                nc.vector.reciprocal(r_t[:], b_t[:])
                F_t = small.tile([8, BLK, 128], f32r, tag="F")
                nc.vector.tensor_mul(F_t[:], y_ps[:], r_t[:])
                fe_ps = pf.tile([128, BLK, 128], f32)
                nc.tensor.matmul(
                    fe_ps[:], ex[:], F_t[:],
                    start=True, stop=True)
                v_t = work.tile([128, BLK, 128], f32, tag="v")
                nc.vector.tensor_mul(v_t[:], psc[:], fe_ps[:])
                nc.sync.dma_start(out=out_v[:, r0:r0 + BLK, :], in_=v_t[:])

    nc.compile()
    return nc


def _prep_in_maps(u, W):
    x = u.reshape(N, CIN, H, W_SP)
    wl = _weight_tiles(W).reshape(128, N_MM * 128)
    bd = _block_diag()
    ex = np.ascontiguousarray(bd.T)
    s = _s_map()
    in_maps = []
    for core in range(N_CORES):
        n, half = core // 2, core % 2
        XA, XC = _inputs_core(x[n], half)
        ss = s[half * 64:(half + 1) * 64, :].reshape(1, ROWS * 128)
        in_maps.append({
            "xa": XA.reshape(128, XROWS * XCOLS),
            "xc": XC.reshape(128, XROWS * XCOLS),
            "wl": wl,
            "bd": bd,
            "ex": ex,
            "ss": np.ascontiguousarray(ss),
        })
    return in_maps


def run(u, W, trace=False):
    """Returns (out [N,T1,Z1,H,W] f32, BassKernelResults)."""
    from concourse.bass_utils import run_bass_kernel_spmd

    if "nc" not in _CACHE:
        _CACHE["nc"] = build_nc()
    nc = _CACHE["nc"]
    in_maps = _prep_in_maps(np.asarray(u, np.float32), np.asarray(W, np.float32))
    res = run_bass_kernel_spmd(nc, in_maps, list(range(N_CORES)), trace=trace)
    out = np.empty((N, T1, Z1, H, W_SP), np.float32)
    for core in range(N_CORES):
        n, half = core // 2, core % 2
        o = res.results[core]["out"].reshape(T1, Z1, ROWS, 128)
        out[n, :, :, half * 64:(half + 1) * 64, :] = o
    return out, res


def kernel(u, W):
    out, _ = run(u, W, trace=False)
    return out
